# revision 1
# baseline (speedup 1.0000x reference)
"""Trainium2 Bass kernel v2 for nn_MultiHeadAttention (B=4, S=2048, H=16, D=64).

Sharding: 8 cores = 4 batches x 2 query-halves (seq-parallel). Each core owns
1024 query rows of one batch, all 16 heads, and produces the FULL 1024 output
columns for its rows — zero cross-core communication (no collective).

Math folds (all exact):
- Both projections folded to the Q side: energy^T = Kraw^T (M Qnat + w 1^T)
  with M = wk^T wq/32, w = wk^T bq/32 — K needs NO on-device projection.
- bk/bq per-query terms drop (softmax shift-invariance per query column).
- V projection folded past attention INTO wo on host: woe = wo @ blockdiag(wv)
  (weights-only), so raw V rides through attention; a ones column in V makes
  row 64 of R = Vnat^T P the softmax denominator.
- v bias: bo_eff = bo + wo @ tile(bv) host-side.

All engine ops sit at partition base 0 (ISA tile-position constraint); the
hidden matrix assembles into 128-partition head-pair blocks via DMA placement
for odd heads so the output GEMM contracts 128 partitions per step.
"""

import numpy as np

try:
    from ml_dtypes import bfloat16 as np_bf16
except ImportError:  # only needed when USE_BF16
    np_bf16 = None

import concourse.bass as bass
import concourse.mybir as mybir
import concourse.tile as tile
from concourse import bacc
from concourse.bass_utils import run_bass_kernel_spmd

f32 = mybir.dt.float32
f32r = mybir.dt.float32r
bf16 = mybir.dt.bfloat16

USE_BF16 = False  # bf16 matmul inputs measured slower (per-mm convert cost)
ENERGY_STOP = False  # stop flag is sim-only per bass docs; test dropping it
MMDT = bf16 if USE_BF16 else f32r

B, S, H, D = 4, 2048, 16, 64
SQ = 1024  # query rows per core
NKB = S // 128  # 16 k-blocks
VW = D + 1  # V block width incl. ones column
EXP = mybir.ActivationFunctionType.Exp


def round_fp32r(x: np.ndarray) -> np.ndarray:
    b = np.ascontiguousarray(x.astype(np.float32)).view(np.uint32)
    return ((b + 0x800) & 0xFFFFF000).view(np.float32)


def to_mmdt(x: np.ndarray) -> np.ndarray:
    if USE_BF16:
        return np.ascontiguousarray(x).astype(np_bf16)
    return round_fp32r(np.ascontiguousarray(x))


def build(reps=1):
    nc = bacc.Bacc("TRN2", target_bir_lowering=False, num_devices=8)

    m32t = nc.dram_tensor("m32t", [D + 1, D], MMDT, kind="ExternalInput")
    qaug = nc.dram_tensor("qaug", [D + 1, H * SQ], MMDT, kind="ExternalInput")
    ktr = nc.dram_tensor("ktr", [H, D, S], MMDT, kind="ExternalInput")
    vna = nc.dram_tensor("vna", [H, 128, NKB * VW], MMDT, kind="ExternalInput")
    woet = nc.dram_tensor("woet", [128, 8192], MMDT, kind="ExternalInput")
    boe = nc.dram_tensor("boe", [128, 8], f32, kind="ExternalInput")
    out = nc.dram_tensor("out", [128, 8192], MMDT, kind="ExternalOutput")

    with tile.TileContext(nc) as tc:
        for r in range(reps):
            _one_rep(nc, tc, m32t, qaug, ktr, vna, woet, boe, out, r)
    nc.compile()
    return nc


def _one_rep(nc, tc, m32t, qaug, ktr, vna, woet, boe, out, r):
    with tc.tile_pool(name=f"keep{r}", bufs=1) as keep:
        On = keep.tile([128, 8192], MMDT, tag="On")

        with tc.tile_pool(name="qkeep", bufs=1) as qkeep:
            Qp = qkeep.tile([D, H * SQ], MMDT, tag="Qp")

            # ---- Phase Q: project Q (+bias fold) for all heads ----
            with (
                tc.tile_pool(name="qraw", bufs=1) as qraw,
                tc.tile_pool(name="psq", bufs=2, space="PSUM") as psq,
            ):
                m32t_s = qraw.tile([D + 1, D], MMDT, tag="m32t")
                nc.default_dma_engine.dma_start(out=m32t_s, in_=m32t[:])
                qaug_s = qraw.tile([D + 1, H * SQ], MMDT, tag="qaug")
                nc.default_dma_engine.dma_start(out=qaug_s, in_=qaug[:])
                for h in range(H):
                    pq = psq.tile([D, 1024], f32, tag="pq")
                    for qc in range(2):
                        nc.tensor.matmul(
                            pq[:, qc * 512 : qc * 512 + 512],
                            lhsT=m32t_s[:],
                            rhs=qaug_s[:, h * SQ + qc * 512 : h * SQ + qc * 512 + 512],
                            start=True,
                            stop=False,
                            skip_group_check=True,
                        )
                    nc.vector.tensor_copy(Qp[:, h * SQ : (h + 1) * SQ], pq[:])

            # ---- Attention per head ----
            with (
                tc.tile_pool(name="kv", bufs=2) as kv,
                tc.tile_pool(name="scp", bufs=2, space="PSUM") as scp,
                tc.tile_pool(name="rp", bufs=2, space="PSUM") as rp,
                tc.tile_pool(name="pex", bufs=2) as pex,
                tc.tile_pool(name="nrm", bufs=2) as nrm,
                tc.tile_pool(name="bdram", bufs=2, space="DRAM") as bdram,
            ):
                for h in range(H):
                    ktr_t = kv.tile([D, S], MMDT, tag="ktr")
                    nc.default_dma_engine.dma_start(out=ktr_t, in_=ktr[h])
                    vna_t = kv.tile([128, NKB * VW], MMDT, tag="vna")
                    nc.default_dma_engine.dma_start(out=vna_t, in_=vna[h])
                    R = rp.tile([VW, 1024], f32, tag="R")
                    for qc in range(2):
                        for kb2 in range(8):
                            sc = scp.tile([128, 1024], f32, tag="sc")
                            for j in range(2):
                                kb = 2 * kb2 + j
                                nc.tensor.matmul(
                                    sc[:, j * 512 : j * 512 + 512],
                                    lhsT=ktr_t[:, kb * 128 : kb * 128 + 128],
                                    rhs=Qp[
                                        :,
                                        h * SQ + qc * 512 : h * SQ + qc * 512 + 512,
                                    ],
                                    start=True,
                                    stop=ENERGY_STOP,
                                    skip_group_check=not ENERGY_STOP,
                                )
                            pt = pex.tile([128, 1024], MMDT, tag="pt")
                            nc.scalar.activation(pt[:], sc[:], EXP, scale=1.0)
                            for j in range(2):
                                kb = 2 * kb2 + j
                                nc.tensor.matmul(
                                    R[:, qc * 512 : qc * 512 + 512],
                                    lhsT=vna_t[:, kb * VW : kb * VW + VW],
                                    rhs=pt[:, j * 512 : j * 512 + 512],
                                    start=(kb == 0),
                                    stop=False,
                                    skip_group_check=True,
                                )
                    # normalize: R[0:64]/R[64] -> On head-pair slot
                    rcp = nrm.tile([1, 1024], f32r, tag="rcp")
                    with nc.allow_low_precision(reason="fp32r softmax denom"):
                        nc.vector.reciprocal(rcp[:], R[D : D + 1, :])
                    bd = bdram.tile([1, 1024], f32r, tag="bd")
                    nc.default_dma_engine.dma_start(out=bd[:], in_=rcp[:])
                    bcs = nrm.tile([D, 1024], f32r, tag="bcs")
                    bd_b = bass.AP(
                        tensor=bd.tensor,
                        offset=bd.offset,
                        ap=[[0, D]] + list(bd.ap)[1:],
                    )
                    nc.default_dma_engine.dma_start(out=bcs[:], in_=bd_b)
                    hp, par = h // 2, h % 2
                    if par == 0:
                        nc.vector.tensor_mul(
                            On[0:D, hp * 1024 : hp * 1024 + 1024], R[0:D, :], bcs[:]
                        )
                    else:
                        tmp = nrm.tile([D, 1024], MMDT, tag="tmp")
                        nc.vector.tensor_mul(tmp[:], R[0:D, :], bcs[:])
                        nc.default_dma_engine.dma_start(
                            out=On[D : 2 * D, hp * 1024 : hp * 1024 + 1024],
                            in_=tmp[:],
                        )

        # ---- Phase C: out = woe^T-blocks @ On + boe ----
        with (
            tc.tile_pool(name="pcw", bufs=1) as pcw,
            tc.tile_pool(name="fin", bufs=1) as finp,
            tc.tile_pool(name="fps", bufs=2, space="PSUM") as fpsp,
        ):
            woet_s = pcw.tile([128, 8192], MMDT, tag="woet")
            nc.default_dma_engine.dma_start(out=woet_s, in_=woet[:])
            boe_s = pcw.tile([128, 8], f32, tag="boe")
            nc.default_dma_engine.dma_start(out=boe_s, in_=boe[:])
            fo = finp.tile([128, 8192], MMDT, tag="fo")
            for ob in range(8):
                fp_ = fpsp.tile([128, 1024], f32, tag="fp")
                for j in range(8):
                    for qc in range(2):
                        nc.tensor.matmul(
                            fp_[:, qc * 512 : qc * 512 + 512],
                            lhsT=woet_s[
                                :, (j * 8 + ob) * 128 : (j * 8 + ob) * 128 + 128
                            ],
                            rhs=On[:, j * 1024 + qc * 512 : j * 1024 + qc * 512 + 512],
                            start=(j == 0),
                            stop=False,
                            skip_group_check=True,
                        )
                nc.vector.tensor_scalar_add(
                    fo[:, ob * 1024 : ob * 1024 + 1024], fp_[:], boe_s[:, ob : ob + 1]
                )
            nc.default_dma_engine.dma_start(out=out[:], in_=fo[:])


_NC_CACHE = {}


def _get_nc(reps=1, use_cc=True):
    key = reps
    if key not in _NC_CACHE:
        _NC_CACHE[key] = build(reps)
    return _NC_CACHE[key]


def _prep_core_inputs(values, keys, query, wq, bq, wk, bk, wv, bv, wo, bo):
    """Build the 8 per-core input maps (host-side shard + layout prep)."""
    del bk  # drops in softmax (per-query constant)
    M = (wk.T.astype(np.float64) @ wq.astype(np.float64)) / 32.0
    w = (wk.T.astype(np.float64) @ bq.astype(np.float64)) / 32.0
    # lhsT for Q projection: rows = [M^T; w^T], so lhsT^T @ [Qnat;1] = M Qnat + w 1^T
    m32t = to_mmdt(np.concatenate([M.T, w.reshape(1, D)], axis=0))

    bv_full = np.tile(bv, H)
    bo_eff = (
        bo.astype(np.float64) + wo.astype(np.float64) @ bv_full.astype(np.float64)
    ).astype(np.float32)
    # fold wv past attention into wo: woe[:, h-block] = wo[:, h-block] @ wv
    woe = (
        wo.astype(np.float64).reshape(1024, H, D) @ wv.astype(np.float64)
    ).reshape(1024, 1024)
    # lhsT tiles for phase C: woet[p, (j*8+ob)*128 + c] = woe[ob*128+c, j*128+p]
    woeT = woe.T.astype(np.float32)  # [hid, out]
    woet = np.empty((128, 8192), np.float32)
    for j in range(8):
        for ob in range(8):
            woet[:, (j * 8 + ob) * 128 : (j * 8 + ob) * 128 + 128] = woeT[
                j * 128 : j * 128 + 128, ob * 128 : ob * 128 + 128
            ]
    woet = to_mmdt(woet)
    boe_l = np.ascontiguousarray(bo_eff.reshape(8, 128).T)  # [128, 8]

    in_maps = []
    for c in range(8):
        b, half = c // 2, c % 2
        rows = slice(half * SQ, (half + 1) * SQ)
        # qaug [65, 16*1024]: cols h*1024+r; partitions: d (then ones row)
        qt = query[b, rows].reshape(SQ, H, D).transpose(1, 2, 0)  # [H, D, SQ]
        qa = np.concatenate([qt, np.ones((H, 1, SQ), np.float32)], axis=1)
        qaug_c = to_mmdt(qa.transpose(1, 0, 2).reshape(D + 1, H * SQ))
        # ktr [H, D, S]
        ktr_c = to_mmdt(keys[b].reshape(S, H, D).transpose(1, 2, 0))
        # vna [H, 128, NKB*VW]: [h, p, kb*VW + c]
        vn = values[b].reshape(NKB, 128, H, D)  # [kb, p, h, d]
        vna_c = np.concatenate(
            [vn, np.ones((NKB, 128, H, 1), np.float32)], axis=3
        ).transpose(2, 1, 0, 3).reshape(H, 128, NKB * VW)
        in_maps.append(
            dict(
                m32t=m32t,
                qaug=np.ascontiguousarray(qaug_c),
                ktr=np.ascontiguousarray(ktr_c),
                vna=to_mmdt(vna_c),
                woet=woet,
                boe=boe_l,
            )
        )
    return in_maps


def kernel(values, keys, query, wq, bq, wk, bk, wv, bv, wo, bo):
    values = np.asarray(values, np.float32)
    keys = np.asarray(keys, np.float32)
    query = np.asarray(query, np.float32)
    in_maps = _prep_core_inputs(
        values, keys, query,
        np.asarray(wq, np.float32), np.asarray(bq, np.float32),
        np.asarray(wk, np.float32), np.asarray(bk, np.float32),
        np.asarray(wv, np.float32), np.asarray(bv, np.float32),
        np.asarray(wo, np.float32), np.asarray(bo, np.float32),
    )
    nc = _get_nc()
    res = run_bass_kernel_spmd(nc, in_maps, list(range(8)))
    out = np.empty((B, S, 1024), np.float32)
    for c in range(8):
        b, half = c // 2, c % 2
        arr = res.results[c]["out"].astype(np.float32).reshape(128, 8, SQ)
        out[b, half * SQ : (half + 1) * SQ, :] = arr.transpose(2, 1, 0).reshape(
            SQ, 1024
        )
    return out



# revision 2
# speedup vs baseline: 43.4935x; 43.4935x over previous
"""Trainium2 Bass kernel v5 for nn_MultiHeadAttention (B=4, S=2048, H=16, D=64).

Sharding: 8 cores = 4 batches x 2 query-halves; no collectives.

v5 structure: ONE outer For_i over reps wrapping the whole per-rep program,
so chained-reps timing prices only the true marginal on-device execution
(program static size is reps-independent). Inside: the Q projection is folded
into the per-head loop (streamed), energy/exp/AV run in 16 waves with two
alternating 2-bank PSUM tiles, and phase C reuses the energy PSUM banks.

Math folds (identical to v2/v3, all exact):
- Energy = Kraw^T (M Qnat + w 1^T), M = wk^T wq/32, w = wk^T bq/32.
- bk drops in softmax; V rides raw; wv folded into wo; ones-column in V makes
  row 64 of R the softmax denominator; bo_eff = bo + wo @ tile(bv).

Constraints honored: matmul lhsT static offsets (weights staged into fixed
tiles); all pool tiles allocated outside For_i bodies.
"""

import numpy as np

import concourse.bass as bass
import concourse.mybir as mybir
import concourse.tile as tile
from concourse import bacc
from concourse.bass_utils import run_bass_kernel_spmd

f32 = mybir.dt.float32
f32r = mybir.dt.float32r

B, S, H, D = 4, 2048, 16, 64
SQ = 1024          # query rows per core
NKB = S // 128     # 16 k-blocks per head
VW = D + 1         # V block width incl. ones column
EXP = mybir.ActivationFunctionType.Exp


def round_fp32r(x: np.ndarray) -> np.ndarray:
    b = np.ascontiguousarray(x.astype(np.float32)).view(np.uint32)
    return ((b + 0x800) & 0xFFFFF000).view(np.float32)


def build(reps=1):
    nc = bacc.Bacc("TRN2", target_bir_lowering=False, num_devices=8)

    m32t = nc.dram_tensor("m32t", [D + 1, D], f32r, kind="ExternalInput")
    qaug = nc.dram_tensor("qaug", [D + 1, H * SQ], f32r, kind="ExternalInput")
    ktr2 = nc.dram_tensor("ktr2", [D, H * S], f32r, kind="ExternalInput")
    vna2 = nc.dram_tensor("vna2", [128, H * NKB * VW], f32r, kind="ExternalInput")
    woec = nc.dram_tensor("woec", [D, 8 * H * 128], f32r, kind="ExternalInput")
    boe = nc.dram_tensor("boe", [128, 8], f32, kind="ExternalInput")
    out = nc.dram_tensor("out", [128, 8192], f32, kind="ExternalOutput")

    with tile.TileContext(nc) as tc:
        with (
            tc.tile_pool(name="sb", bufs=1) as sb,
            tc.tile_pool(name="ps", bufs=1, space="PSUM") as ps,
            tc.tile_pool(name="bdram", bufs=1, space="DRAM") as bdram,
        ):
            m32t_s = sb.tile([D + 1, D], f32r, tag="m32t")
            boe_s = sb.tile([128, 8], f32, tag="boe")
            qh = sb.tile([D + 1, SQ], f32r, tag="qh")
            Qph = sb.tile([D, SQ], f32r, tag="Qph")
            kh = sb.tile([D, S], f32r, tag="kh")
            vh = sb.tile([128, NKB * VW], f32r, tag="vh")
            pt_a = sb.tile([128, SQ], f32r, tag="pt_a")
            pt_b = sb.tile([128, SQ], f32r, tag="pt_b")
            rc = sb.tile([1, SQ], f32r, tag="rc")
            bcs = sb.tile([D, SQ], f32r, tag="bcs")
            On2 = sb.tile([D, H * SQ], f32r, tag="On2")
            wstg = sb.tile([D, H * 128], f32r, tag="wstg")
            fo = sb.tile([128, SQ], f32, tag="fo")
            bd = bdram.tile([1, SQ], f32r, tag="bd")
            bd_b = bass.AP(
                tensor=bd.tensor,
                offset=bd.offset,
                ap=[[0, D]] + list(bd.ap)[1:],
            )
            sc_a = ps.tile([128, SQ], f32, tag="sc_a")
            sc_b = ps.tile([128, SQ], f32, tag="sc_b")
            R = ps.tile([VW, SQ], f32, tag="R")
            pq = ps.tile([D, SQ], f32, tag="pq")

            with tc.For_i(0, reps, 1):
                nc.default_dma_engine.dma_start(out=m32t_s, in_=m32t[:])
                nc.default_dma_engine.dma_start(out=boe_s, in_=boe[:])

                # ---- per-head: Q-proj + energy/exp/AV + normalize ----
                with tc.For_i(0, H, 1) as i:
                    nc.default_dma_engine.dma_start(
                        out=qh[:], in_=qaug[:, bass.ds(i * SQ, SQ)]
                    )
                    nc.default_dma_engine.dma_start(
                        out=kh[:], in_=ktr2[:, bass.ds(i * S, S)]
                    )
                    nc.default_dma_engine.dma_start(
                        out=vh[:], in_=vna2[:, bass.ds(i * (NKB * VW), NKB * VW)]
                    )
                    for qc in range(2):
                        nc.tensor.matmul(
                            pq[:, qc * 512 : qc * 512 + 512],
                            lhsT=m32t_s[:],
                            rhs=qh[:, qc * 512 : qc * 512 + 512],
                            start=True,
                            stop=False,
                            skip_group_check=True,
                        )
                    nc.vector.tensor_copy(Qph[:], pq[:])
                    for kb in range(NKB):
                        sc = sc_a if kb % 2 == 0 else sc_b
                        pt = pt_a if kb % 2 == 0 else pt_b
                        for qc in range(2):
                            nc.tensor.matmul(
                                sc[:, qc * 512 : qc * 512 + 512],
                                lhsT=kh[:, kb * 128 : kb * 128 + 128],
                                rhs=Qph[:, qc * 512 : qc * 512 + 512],
                                start=True,
                                stop=False,
                                skip_group_check=True,
                            )
                        nc.scalar.activation(pt[:], sc[:], EXP, scale=1.0)
                        for qc in range(2):
                            nc.tensor.matmul(
                                R[:, qc * 512 : qc * 512 + 512],
                                lhsT=vh[:, kb * VW : kb * VW + VW],
                                rhs=pt[:, qc * 512 : qc * 512 + 512],
                                start=(kb == 0),
                                stop=False,
                                skip_group_check=True,
                            )
                    with nc.allow_low_precision(reason="fp32r softmax denom"):
                        nc.vector.reciprocal(rc[:], R[D : D + 1, :])
                    nc.default_dma_engine.dma_start(out=bd[:], in_=rc[:])
                    nc.default_dma_engine.dma_start(out=bcs[:], in_=bd_b)
                    nc.vector.tensor_mul(
                        On2[:, bass.ds(i * SQ, SQ)], R[0:D, :], bcs[:]
                    )

                # ---- Phase C: out = sum_h woe_h^T @ On2_h + boe ----
                # fp reuses sc_a's PSUM banks (sequential, Tile-serialized).
                with tc.For_i(0, 8, 1) as i:
                    nc.default_dma_engine.dma_start(
                        out=wstg[:], in_=woec[:, bass.ds(i * (H * 128), H * 128)]
                    )
                    for h in range(H):
                        for qc in range(2):
                            nc.tensor.matmul(
                                sc_a[:, qc * 512 : qc * 512 + 512],
                                lhsT=wstg[:, h * 128 : h * 128 + 128],
                                rhs=On2[:, h * SQ + qc * 512 : h * SQ + qc * 512 + 512],
                                start=(h == 0),
                                stop=False,
                                skip_group_check=True,
                            )
                    nc.vector.tensor_scalar_add(
                        fo[:], sc_a[:], boe_s[:, bass.ds(i, 1)]
                    )
                    nc.default_dma_engine.dma_start(
                        out=out[:, bass.ds(i * SQ, SQ)], in_=fo[:]
                    )
    nc.compile()
    return nc


_NC_CACHE = {}


def _get_nc(reps=1):
    if reps not in _NC_CACHE:
        _NC_CACHE[reps] = build(reps)
    return _NC_CACHE[reps]


def _prep_core_inputs(values, keys, query, wq, bq, wk, bk, wv, bv, wo, bo):
    """Build the 8 per-core input maps (host-side shard + layout prep)."""
    del bk  # drops in softmax (per-query constant)
    M = (wk.T.astype(np.float64) @ wq.astype(np.float64)) / 32.0
    w = (wk.T.astype(np.float64) @ bq.astype(np.float64)) / 32.0
    m32t = round_fp32r(np.concatenate([M.T, w.reshape(1, D)], axis=0))

    bv_full = np.tile(bv, H)
    bo_eff = (
        bo.astype(np.float64) + wo.astype(np.float64) @ bv_full.astype(np.float64)
    ).astype(np.float32)
    woe = (
        wo.astype(np.float64).reshape(1024, H, D) @ wv.astype(np.float64)
    ).reshape(1024, 1024)
    woeT = woe.T.astype(np.float32)  # [hid, out]
    # woec[d, ob*2048 + h*128 + p] = woe[ob*128+p, h*64+d]
    woec = np.empty((D, 8 * H * 128), np.float32)
    for ob in range(8):
        for h in range(H):
            woec[:, ob * 2048 + h * 128 : ob * 2048 + h * 128 + 128] = woeT[
                h * D : (h + 1) * D, ob * 128 : ob * 128 + 128
            ]
    woec = round_fp32r(woec)
    boe_l = np.ascontiguousarray(bo_eff.reshape(8, 128).T)  # [128, 8]

    in_maps = []
    for c in range(8):
        b, half = c // 2, c % 2
        rows = slice(half * SQ, (half + 1) * SQ)
        qt = query[b, rows].reshape(SQ, H, D).transpose(1, 2, 0)  # [H, D, SQ]
        qa = np.concatenate([qt, np.ones((H, 1, SQ), np.float32)], axis=1)
        qaug_c = round_fp32r(qa.transpose(1, 0, 2).reshape(D + 1, H * SQ))
        ktr_c = round_fp32r(
            keys[b].reshape(S, H, D).transpose(2, 1, 0).reshape(D, H * S)
        )
        vn = values[b].reshape(NKB, 128, H, D)  # [kb, p, h, d]
        vna_c = (
            np.concatenate([vn, np.ones((NKB, 128, H, 1), np.float32)], axis=3)
            .transpose(1, 2, 0, 3)
            .reshape(128, H * NKB * VW)
        )
        in_maps.append(
            dict(
                m32t=m32t,
                qaug=np.ascontiguousarray(qaug_c),
                ktr2=np.ascontiguousarray(ktr_c),
                vna2=round_fp32r(np.ascontiguousarray(vna_c)),
                woec=woec,
                boe=boe_l,
            )
        )
    return in_maps


def kernel(values, keys, query, wq, bq, wk, bk, wv, bv, wo, bo):
    values = np.asarray(values, np.float32)
    keys = np.asarray(keys, np.float32)
    query = np.asarray(query, np.float32)
    in_maps = _prep_core_inputs(
        values, keys, query,
        np.asarray(wq, np.float32), np.asarray(bq, np.float32),
        np.asarray(wk, np.float32), np.asarray(bk, np.float32),
        np.asarray(wv, np.float32), np.asarray(bv, np.float32),
        np.asarray(wo, np.float32), np.asarray(bo, np.float32),
    )
    nc = _get_nc()
    res = run_bass_kernel_spmd(nc, in_maps, list(range(8)))
    out = np.empty((B, S, 1024), np.float32)
    for c in range(8):
        b, half = c // 2, c % 2
        arr = res.results[c]["out"].astype(np.float32).reshape(128, 8, SQ)
        out[b, half * SQ : (half + 1) * SQ, :] = arr.transpose(2, 1, 0).reshape(
            SQ, 1024
        )
    return out


# revision 3
# speedup vs baseline: 78.2888x; 1.8000x over previous
"""v6: outer For_i over reps + fully STATIC per-rep body (no inner loops).

Static program size doesn't scale with reps (outer loop), so the body is
fully unrolled with static addresses, letting Tile double-buffer freely with
no inner back-edge barriers. Normalization is deferred: per head only
(a) DVE copy R[0:64] -> On2 slice (unnormalized), (b) DMA R[64] -> denall[h];
after the head sweep one reciprocal + broadcast + 4 in-place muls normalize
On2. PSUM: sc pairs [128,1536] (3 banks each) + one R [65,1024] (2 banks).
"""

import numpy as np

import concourse.bass as bass
import concourse.mybir as mybir
import concourse.tile as tile
from concourse import bacc
from concourse.bass_utils import run_bass_kernel_spmd

f32 = mybir.dt.float32
f32r = mybir.dt.float32r

B, S, H, D = 4, 2048, 16, 64
SQ = 1024
NKB = S // 128
VW = D + 1
EXP = mybir.ActivationFunctionType.Exp


def round_fp32r(x: np.ndarray) -> np.ndarray:
    b = np.ascontiguousarray(x.astype(np.float32)).view(np.uint32)
    return ((b + 0x800) & 0xFFFFF000).view(np.float32)


def build(reps=1):
    nc = bacc.Bacc("TRN2", target_bir_lowering=False, num_devices=8)

    m32t = nc.dram_tensor("m32t", [D + 1, D], f32r, kind="ExternalInput")
    qaug = nc.dram_tensor("qaug", [D + 1, H * SQ], f32r, kind="ExternalInput")
    ktr2 = nc.dram_tensor("ktr2", [D, H * S], f32r, kind="ExternalInput")
    vna2 = nc.dram_tensor("vna2", [128, H * NKB * VW], f32r, kind="ExternalInput")
    woec = nc.dram_tensor("woec", [D, 8 * H * 128], f32r, kind="ExternalInput")
    boe = nc.dram_tensor("boe", [128, 8], f32, kind="ExternalInput")
    out = nc.dram_tensor("out", [128, 8192], f32, kind="ExternalOutput")

    with tile.TileContext(nc) as tc:
        with (
            tc.tile_pool(name="sb", bufs=1) as sb,
            tc.tile_pool(name="ps", bufs=1, space="PSUM") as ps,
            tc.tile_pool(name="bdram", bufs=1, space="DRAM") as bdram,
        ):
            m32t_s = sb.tile([D + 1, D], f32r, tag="m32t")
            boe_s = sb.tile([128, 8], f32, tag="boe")
            Qp = sb.tile([D, H * SQ], f32r, tag="Qp")
            On2 = sb.tile([D, H * SQ], f32r, tag="On2")
            qh0 = sb.tile([D + 1, SQ], f32r, tag="qh0")
            qh = [qh0, qh0]
            kh0 = sb.tile([D, S], f32r, tag="kh0")
            kh1 = sb.tile([D, S], f32r, tag="kh1")
            kh = [kh0, kh1]
            vh = sb.tile([128, NKB * VW], f32r, tag="vh")
            pt0 = sb.tile([128, 1536], f32r, tag="pt0")
            pt1 = sb.tile([128, 1536], f32r, tag="pt1")
            pt = [pt0, pt1]
            denall = sb.tile([16, SQ], f32, tag="denall")
            rcrow = sb.tile([1, SQ], f32, tag="rcrow")
            denr = sb.tile([16, SQ], f32r, tag="denr")
            bcs4 = sb.tile([D, 4 * SQ], f32r, tag="bcs4")
            wstg = sb.tile([D, H * 128], f32r, tag="wstg")
            fo = sb.tile([128, SQ], f32, tag="fo")
            bd16 = bdram.tile([16, SQ], f32r, tag="bd16")

            sc0 = ps.tile([128, 1536], f32, tag="sc0")
            sc1 = ps.tile([128, 1536], f32, tag="sc1")
            sc = [sc0, sc1]
            R = ps.tile([VW, SQ], f32, tag="R")

            with tc.For_i(0, reps, 1):
                nc.default_dma_engine.dma_start(out=m32t_s, in_=m32t[:])
                nc.default_dma_engine.dma_start(out=boe_s, in_=boe[:])

                # ---- Q projection (uses sc banks as scratch psum) ----
                for h in range(H):
                    x = h % 2
                    nc.default_dma_engine.dma_start(
                        out=qh[x], in_=qaug[:, h * SQ : (h + 1) * SQ]
                    )
                    for qc in range(2):
                        nc.tensor.matmul(
                            sc[x][0:D, qc * 512 : qc * 512 + 512],
                            lhsT=m32t_s[:],
                            rhs=qh[x][:, qc * 512 : qc * 512 + 512],
                            start=True,
                            stop=False,
                            skip_group_check=True,
                        )
                    nc.vector.tensor_copy(
                        Qp[:, h * SQ : (h + 1) * SQ], sc[x][0:D, 0:SQ]
                    )

                # ---- heads: energy/exp/AV, unnormalized R -> On2 + den ----
                for h in range(H):
                    x = h % 2
                    nc.default_dma_engine.dma_start(
                        out=kh[x], in_=ktr2[:, h * S : (h + 1) * S]
                    )
                    nc.default_dma_engine.dma_start(
                        out=vh, in_=vna2[:, h * (NKB * VW) : (h + 1) * (NKB * VW)]
                    )
                    # 32 units (kb, qc); waves of 3 units into 3-bank sc pairs
                    units = [(kb, qc) for kb in range(NKB) for qc in range(2)]
                    w = 0
                    while w * 3 < 32:
                        chunk = units[w * 3 : w * 3 + 3]
                        y = w % 2
                        for s_i, (kb, qc) in enumerate(chunk):
                            nc.tensor.matmul(
                                sc[y][:, s_i * 512 : s_i * 512 + 512],
                                lhsT=kh[x][:, kb * 128 : kb * 128 + 128],
                                rhs=Qp[:, h * SQ + qc * 512 : h * SQ + qc * 512 + 512],
                                start=True,
                                stop=False,
                                skip_group_check=True,
                            )
                        nn = len(chunk) * 512
                        nc.scalar.activation(
                            pt[y][:, 0:nn], sc[y][:, 0:nn], EXP, scale=1.0
                        )
                        for s_i, (kb, qc) in enumerate(chunk):
                            nc.tensor.matmul(
                                R[:, qc * 512 : qc * 512 + 512],
                                lhsT=vh[:, kb * VW : kb * VW + VW],
                                rhs=pt[y][:, s_i * 512 : s_i * 512 + 512],
                                start=(kb == 0),
                                stop=False,
                                skip_group_check=True,
                            )
                        w += 1
                    # unnormalized evacuation + denominator row
                    # (DMA can't read PSUM: stage the den row through SBUF)
                    nc.vector.tensor_copy(
                        On2[:, h * SQ : (h + 1) * SQ], R[0:D, :]
                    )
                    nc.vector.tensor_copy(rcrow[:], R[D : D + 1, :])
                    nc.default_dma_engine.dma_start(
                        out=denall[h : h + 1, :], in_=rcrow[:]
                    )

                # ---- deferred normalization of On2 (in 4 chunks) ----
                with nc.allow_low_precision(reason="fp32r softmax denom"):
                    nc.vector.reciprocal(denr[:], denall[:])
                nc.default_dma_engine.dma_start(out=bd16[:], in_=denr[:])
                bd_ap = list(bd16.ap)
                rs = bd_ap[0][0]  # row stride of the DRAM tile, in AP units
                for c in range(4):
                    src = bass.AP(
                        tensor=bd16.tensor,
                        offset=bd16.offset + c * 4 * rs,
                        ap=[[0, D], [rs, 4], list(bd_ap[1])],
                    )
                    nc.default_dma_engine.dma_start(out=bcs4[:], in_=src)
                    nc.vector.tensor_mul(
                        On2[:, c * 4 * SQ : (c + 1) * 4 * SQ],
                        On2[:, c * 4 * SQ : (c + 1) * 4 * SQ],
                        bcs4[:],
                    )

                # ---- Phase C (fp reuses sc bank pairs) ----
                for ob in range(8):
                    y = ob % 2
                    nc.default_dma_engine.dma_start(
                        out=wstg, in_=woec[:, ob * (H * 128) : (ob + 1) * (H * 128)]
                    )
                    for h in range(H):
                        for qc in range(2):
                            nc.tensor.matmul(
                                sc[y][:, qc * 512 : qc * 512 + 512],
                                lhsT=wstg[:, h * 128 : h * 128 + 128],
                                rhs=On2[:, h * SQ + qc * 512 : h * SQ + qc * 512 + 512],
                                start=(h == 0),
                                stop=False,
                                skip_group_check=True,
                            )
                    nc.vector.tensor_scalar_add(
                        fo[:], sc[y][:, 0:SQ], boe_s[:, ob : ob + 1]
                    )
                    nc.default_dma_engine.dma_start(
                        out=out[:, ob * SQ : (ob + 1) * SQ], in_=fo[:]
                    )
    nc.compile()
    return nc


_NC_CACHE = {}


def _get_nc(reps=1):
    if reps not in _NC_CACHE:
        _NC_CACHE[reps] = build(reps)
    return _NC_CACHE[reps]


def _prep_core_inputs(values, keys, query, wq, bq, wk, bk, wv, bv, wo, bo):
    del bk
    M = (wk.T.astype(np.float64) @ wq.astype(np.float64)) / 32.0
    w = (wk.T.astype(np.float64) @ bq.astype(np.float64)) / 32.0
    m32t = round_fp32r(np.concatenate([M.T, w.reshape(1, D)], axis=0))

    bv_full = np.tile(bv, H)
    bo_eff = (
        bo.astype(np.float64) + wo.astype(np.float64) @ bv_full.astype(np.float64)
    ).astype(np.float32)
    woe = (
        wo.astype(np.float64).reshape(1024, H, D) @ wv.astype(np.float64)
    ).reshape(1024, 1024)
    woeT = woe.T.astype(np.float32)
    woec = np.empty((D, 8 * H * 128), np.float32)
    for ob in range(8):
        for h in range(H):
            woec[:, ob * 2048 + h * 128 : ob * 2048 + h * 128 + 128] = woeT[
                h * D : (h + 1) * D, ob * 128 : ob * 128 + 128
            ]
    woec = round_fp32r(woec)
    boe_l = np.ascontiguousarray(bo_eff.reshape(8, 128).T)

    in_maps = []
    for c in range(8):
        b, half = c // 2, c % 2
        rows = slice(half * SQ, (half + 1) * SQ)
        qt = query[b, rows].reshape(SQ, H, D).transpose(1, 2, 0)
        qa = np.concatenate([qt, np.ones((H, 1, SQ), np.float32)], axis=1)
        qaug_c = round_fp32r(qa.transpose(1, 0, 2).reshape(D + 1, H * SQ))
        ktr_c = round_fp32r(
            keys[b].reshape(S, H, D).transpose(2, 1, 0).reshape(D, H * S)
        )
        vn = values[b].reshape(NKB, 128, H, D)
        vna_c = (
            np.concatenate([vn, np.ones((NKB, 128, H, 1), np.float32)], axis=3)
            .transpose(1, 2, 0, 3)
            .reshape(128, H * NKB * VW)
        )
        in_maps.append(
            dict(
                m32t=m32t,
                qaug=np.ascontiguousarray(qaug_c),
                ktr2=np.ascontiguousarray(ktr_c),
                vna2=round_fp32r(np.ascontiguousarray(vna_c)),
                woec=woec,
                boe=boe_l,
            )
        )
    return in_maps


def kernel(values, keys, query, wq, bq, wk, bk, wv, bv, wo, bo):
    values = np.asarray(values, np.float32)
    keys = np.asarray(keys, np.float32)
    query = np.asarray(query, np.float32)
    in_maps = _prep_core_inputs(
        values, keys, query,
        np.asarray(wq, np.float32), np.asarray(bq, np.float32),
        np.asarray(wk, np.float32), np.asarray(bk, np.float32),
        np.asarray(wv, np.float32), np.asarray(bv, np.float32),
        np.asarray(wo, np.float32), np.asarray(bo, np.float32),
    )
    nc = _get_nc()
    res = run_bass_kernel_spmd(nc, in_maps, list(range(8)))
    out = np.empty((B, S, 1024), np.float32)
    for c in range(8):
        b, half = c // 2, c % 2
        arr = res.results[c]["out"].astype(np.float32).reshape(128, 8, SQ)
        out[b, half * SQ : (half + 1) * SQ, :] = arr.transpose(2, 1, 0).reshape(
            SQ, 1024
        )
    return out


# revision 4
# speedup vs baseline: 143.9107x; 1.8382x over previous
"""v7: v6 + Q-projection folded into the head loop (frees the 64KB Qp tile)
and the freed SBUF spent double-buffering vh/wstg/qh for DMA/compute overlap.

Static program size doesn't scale with reps (outer loop), so the body is
fully unrolled with static addresses, letting Tile double-buffer freely with
no inner back-edge barriers. Normalization is deferred: per head only
(a) DVE copy R[0:64] -> On2 slice (unnormalized), (b) DMA R[64] -> denall[h];
after the head sweep one reciprocal + broadcast + 4 in-place muls normalize
On2. PSUM: sc pairs [128,1536] (3 banks each) + one R [65,1024] (2 banks).
"""

import numpy as np

import concourse.bass as bass
import concourse.mybir as mybir
import concourse.tile as tile
from concourse import bacc
from concourse.bass_utils import run_bass_kernel_spmd

f32 = mybir.dt.float32
f32r = mybir.dt.float32r

B, S, H, D = 4, 2048, 16, 64
SQ = 1024
NKB = S // 128
VW = D + 1
EXP = mybir.ActivationFunctionType.Exp


def round_fp32r(x: np.ndarray) -> np.ndarray:
    b = np.ascontiguousarray(x.astype(np.float32)).view(np.uint32)
    return ((b + 0x800) & 0xFFFFF000).view(np.float32)


def build(reps=1):
    nc = bacc.Bacc("TRN2", target_bir_lowering=False, num_devices=8)

    m32t = nc.dram_tensor("m32t", [D + 1, D], f32r, kind="ExternalInput")
    qaug = nc.dram_tensor("qaug", [D + 1, H * SQ], f32r, kind="ExternalInput")
    ktr2 = nc.dram_tensor("ktr2", [D, H * S], f32r, kind="ExternalInput")
    vna2 = nc.dram_tensor("vna2", [128, H * NKB * VW], f32r, kind="ExternalInput")
    woec = nc.dram_tensor("woec", [D, 8 * H * 128], f32r, kind="ExternalInput")
    boe = nc.dram_tensor("boe", [128, 8], f32, kind="ExternalInput")
    out = nc.dram_tensor("out", [128, 8192], f32, kind="ExternalOutput")

    with tile.TileContext(nc) as tc:
        with (
            tc.tile_pool(name="sb", bufs=1) as sb,
            tc.tile_pool(name="ps", bufs=1, space="PSUM") as ps,
            tc.tile_pool(name="bdram", bufs=1, space="DRAM") as bdram,
        ):
            m32t_s = sb.tile([D + 1, D], f32r, tag="m32t")
            boe_s = sb.tile([128, 8], f32, tag="boe")
            On2 = sb.tile([D, H * SQ], f32r, tag="On2")
            qh0 = sb.tile([D + 1, SQ], f32r, tag="qh0")
            qh1 = sb.tile([D + 1, SQ], f32r, tag="qh1")
            qh = [qh0, qh1]
            qp0 = sb.tile([D, SQ], f32r, tag="qp0")
            qp1 = sb.tile([D, SQ], f32r, tag="qp1")
            Qph = [qp0, qp1]
            kh0 = sb.tile([D, S], f32r, tag="kh0")
            kh1 = sb.tile([D, S], f32r, tag="kh1")
            kh = [kh0, kh1]
            vh0 = sb.tile([128, NKB * VW], f32r, tag="vh0")
            vh1 = sb.tile([128, NKB * VW], f32r, tag="vh1")
            vh = [vh0, vh1]
            pt0 = sb.tile([128, 1536], f32r, tag="pt0")
            pt1 = sb.tile([128, 1536], f32r, tag="pt1")
            pt = [pt0, pt1]
            denall = sb.tile([16, SQ], f32, tag="denall")
            rcrow = sb.tile([1, SQ], f32, tag="rcrow")
            denr = sb.tile([16, SQ], f32r, tag="denr")
            bcs4 = sb.tile([D, 4 * SQ], f32r, tag="bcs4")
            ws0 = sb.tile([D, H * 128], f32r, tag="ws0")
            ws1 = sb.tile([D, H * 128], f32r, tag="ws1")
            wstg = [ws0, ws1]
            fo = sb.tile([128, SQ], f32, tag="fo")
            bd16 = bdram.tile([16, SQ], f32r, tag="bd16")

            sc0 = ps.tile([128, 1536], f32, tag="sc0")
            sc1 = ps.tile([128, 1536], f32, tag="sc1")
            sc = [sc0, sc1]
            R = ps.tile([VW, SQ], f32, tag="R")

            with tc.For_i(0, reps, 1):
                nc.default_dma_engine.dma_start(out=m32t_s, in_=m32t[:])
                nc.default_dma_engine.dma_start(out=boe_s, in_=boe[:])

                # ---- heads: energy/exp/AV, unnormalized R -> On2 + den ----
                for h in range(H):
                    x = h % 2
                    nc.default_dma_engine.dma_start(
                        out=qh[x], in_=qaug[:, h * SQ : (h + 1) * SQ]
                    )
                    nc.default_dma_engine.dma_start(
                        out=kh[x], in_=ktr2[:, h * S : (h + 1) * S]
                    )
                    nc.default_dma_engine.dma_start(
                        out=vh[x], in_=vna2[:, h * (NKB * VW) : (h + 1) * (NKB * VW)]
                    )
                    # in-head Q projection (sc[x] partitions 0-63 as scratch)
                    for qc in range(2):
                        nc.tensor.matmul(
                            sc[x][0:D, qc * 512 : qc * 512 + 512],
                            lhsT=m32t_s[:],
                            rhs=qh[x][:, qc * 512 : qc * 512 + 512],
                            start=True,
                            stop=False,
                            skip_group_check=True,
                        )
                    nc.vector.tensor_copy(Qph[x][:], sc[x][0:D, 0:SQ])
                    # 32 units (kb, qc); waves of 3 units into 3-bank sc pairs
                    units = [(kb, qc) for kb in range(NKB) for qc in range(2)]
                    w = 0
                    while w * 3 < 32:
                        chunk = units[w * 3 : w * 3 + 3]
                        y = w % 2
                        for s_i, (kb, qc) in enumerate(chunk):
                            nc.tensor.matmul(
                                sc[y][:, s_i * 512 : s_i * 512 + 512],
                                lhsT=kh[x][:, kb * 128 : kb * 128 + 128],
                                rhs=Qph[x][:, qc * 512 : qc * 512 + 512],
                                start=True,
                                stop=False,
                                skip_group_check=True,
                            )
                        nn = len(chunk) * 512
                        nc.scalar.activation(
                            pt[y][:, 0:nn], sc[y][:, 0:nn], EXP, scale=1.0
                        )
                        for s_i, (kb, qc) in enumerate(chunk):
                            nc.tensor.matmul(
                                R[:, qc * 512 : qc * 512 + 512],
                                lhsT=vh[x][:, kb * VW : kb * VW + VW],
                                rhs=pt[y][:, s_i * 512 : s_i * 512 + 512],
                                start=(kb == 0),
                                stop=False,
                                skip_group_check=True,
                            )
                        w += 1
                    # unnormalized evacuation + denominator row
                    # (DMA can't read PSUM: stage the den row through SBUF)
                    nc.vector.tensor_copy(
                        On2[:, h * SQ : (h + 1) * SQ], R[0:D, :]
                    )
                    nc.vector.tensor_copy(rcrow[:], R[D : D + 1, :])
                    nc.default_dma_engine.dma_start(
                        out=denall[h : h + 1, :], in_=rcrow[:]
                    )

                # ---- deferred normalization of On2 (in 4 chunks) ----
                with nc.allow_low_precision(reason="fp32r softmax denom"):
                    nc.vector.reciprocal(denr[:], denall[:])
                nc.default_dma_engine.dma_start(out=bd16[:], in_=denr[:])
                bd_ap = list(bd16.ap)
                rs = bd_ap[0][0]  # row stride of the DRAM tile, in AP units
                for c in range(4):
                    src = bass.AP(
                        tensor=bd16.tensor,
                        offset=bd16.offset + c * 4 * rs,
                        ap=[[0, D], [rs, 4], list(bd_ap[1])],
                    )
                    nc.default_dma_engine.dma_start(out=bcs4[:], in_=src)
                    nc.vector.tensor_mul(
                        On2[:, c * 4 * SQ : (c + 1) * 4 * SQ],
                        On2[:, c * 4 * SQ : (c + 1) * 4 * SQ],
                        bcs4[:],
                    )

                # ---- Phase C (fp reuses sc bank pairs) ----
                for ob in range(8):
                    y = ob % 2
                    nc.default_dma_engine.dma_start(
                        out=wstg[y], in_=woec[:, ob * (H * 128) : (ob + 1) * (H * 128)]
                    )
                    for h in range(H):
                        for qc in range(2):
                            nc.tensor.matmul(
                                sc[y][:, qc * 512 : qc * 512 + 512],
                                lhsT=wstg[y][:, h * 128 : h * 128 + 128],
                                rhs=On2[:, h * SQ + qc * 512 : h * SQ + qc * 512 + 512],
                                start=(h == 0),
                                stop=False,
                                skip_group_check=True,
                            )
                    nc.vector.tensor_scalar_add(
                        fo[:], sc[y][:, 0:SQ], boe_s[:, ob : ob + 1]
                    )
                    nc.default_dma_engine.dma_start(
                        out=out[:, ob * SQ : (ob + 1) * SQ], in_=fo[:]
                    )
    nc.compile()
    return nc


_NC_CACHE = {}


def _get_nc(reps=1):
    if reps not in _NC_CACHE:
        _NC_CACHE[reps] = build(reps)
    return _NC_CACHE[reps]


def _prep_core_inputs(values, keys, query, wq, bq, wk, bk, wv, bv, wo, bo):
    del bk
    M = (wk.T.astype(np.float64) @ wq.astype(np.float64)) / 32.0
    w = (wk.T.astype(np.float64) @ bq.astype(np.float64)) / 32.0
    m32t = round_fp32r(np.concatenate([M.T, w.reshape(1, D)], axis=0))

    bv_full = np.tile(bv, H)
    bo_eff = (
        bo.astype(np.float64) + wo.astype(np.float64) @ bv_full.astype(np.float64)
    ).astype(np.float32)
    woe = (
        wo.astype(np.float64).reshape(1024, H, D) @ wv.astype(np.float64)
    ).reshape(1024, 1024)
    woeT = woe.T.astype(np.float32)
    woec = np.empty((D, 8 * H * 128), np.float32)
    for ob in range(8):
        for h in range(H):
            woec[:, ob * 2048 + h * 128 : ob * 2048 + h * 128 + 128] = woeT[
                h * D : (h + 1) * D, ob * 128 : ob * 128 + 128
            ]
    woec = round_fp32r(woec)
    boe_l = np.ascontiguousarray(bo_eff.reshape(8, 128).T)

    in_maps = []
    for c in range(8):
        b, half = c // 2, c % 2
        rows = slice(half * SQ, (half + 1) * SQ)
        qt = query[b, rows].reshape(SQ, H, D).transpose(1, 2, 0)
        qa = np.concatenate([qt, np.ones((H, 1, SQ), np.float32)], axis=1)
        qaug_c = round_fp32r(qa.transpose(1, 0, 2).reshape(D + 1, H * SQ))
        ktr_c = round_fp32r(
            keys[b].reshape(S, H, D).transpose(2, 1, 0).reshape(D, H * S)
        )
        vn = values[b].reshape(NKB, 128, H, D)
        vna_c = (
            np.concatenate([vn, np.ones((NKB, 128, H, 1), np.float32)], axis=3)
            .transpose(1, 2, 0, 3)
            .reshape(128, H * NKB * VW)
        )
        in_maps.append(
            dict(
                m32t=m32t,
                qaug=np.ascontiguousarray(qaug_c),
                ktr2=np.ascontiguousarray(ktr_c),
                vna2=round_fp32r(np.ascontiguousarray(vna_c)),
                woec=woec,
                boe=boe_l,
            )
        )
    return in_maps


def kernel(values, keys, query, wq, bq, wk, bk, wv, bv, wo, bo):
    values = np.asarray(values, np.float32)
    keys = np.asarray(keys, np.float32)
    query = np.asarray(query, np.float32)
    in_maps = _prep_core_inputs(
        values, keys, query,
        np.asarray(wq, np.float32), np.asarray(bq, np.float32),
        np.asarray(wk, np.float32), np.asarray(bk, np.float32),
        np.asarray(wv, np.float32), np.asarray(bv, np.float32),
        np.asarray(wo, np.float32), np.asarray(bo, np.float32),
    )
    nc = _get_nc()
    res = run_bass_kernel_spmd(nc, in_maps, list(range(8)))
    out = np.empty((B, S, 1024), np.float32)
    for c in range(8):
        b, half = c // 2, c % 2
        arr = res.results[c]["out"].astype(np.float32).reshape(128, 8, SQ)
        out[b, half * SQ : (half + 1) * SQ, :] = arr.transpose(2, 1, 0).reshape(
            SQ, 1024
        )
    return out
